# revision 26
# baseline (speedup 1.0000x reference)
"""Fused ReLU + 4x RMSNorm + 3x (matmul + residual-add) kernel for TRN2.

Reference computation (per token row t, hidden dim H=2048):
    x1 = relu(x); resid = x1
    for s in 0..2:
        y = rmsnorm(resid, g_s)                # norm over H
        resid = y @ W_s + resid
    out = rmsnorm(resid, g3)

Sharding: pure data-parallel over the token dim (32768 tokens -> 8 cores x
4096 tokens). Each row's computation is independent, so no collectives are
needed; W/g are replicated per core.

Per-core design (SPMD, same program on all 8 cores):
  - resid lives in SBUF token-major [128 tok, NT, H] fp32, in blocks of TB
    tokens. Norm reductions are free-dim reductions.
  - matmul operands are bf16: yhat = (resid*rs) cast to bf16, then
    DMA-xbar-transposed (SBUF->SBUF, per 128-token tile) into H-major layout
    for the TensorE stationary operand. W' streams from HBM in [H, 512]
    column blocks. PSUM accumulates fp32 over 16 k-chunks; the psum tile is
    added into resid directly (DVE), keeping the residual chain fp32.
  - norm sums-of-squares accumulate incrementally (ACT Square right after
    each psum-add finalizes a resid slice); each token tile's next-stage
    chain (reduce -> rsqrt -> bf16 cast -> transpose) is emitted inside the
    matmul loop at that tile's final column block, and the next block's
    stage-0 boundary is emitted before the current block's stage-2 matmuls,
    so TensorE operands are always ready ahead of the running phase.
"""

import sys

import numpy as np

try:
    import concourse.bass as bass  # noqa: F401
except ImportError:  # pragma: no cover
    sys.path.insert(0, "/opt/trn_rl_repo")

import concourse.bass as bass
import concourse.tile as tile
from concourse import bacc, mybir
from concourse.bass_utils import run_bass_kernel_spmd

import ml_dtypes

EPS = 1e-6
TOKENS = 32768
HIDDEN = 2048
N_CORES = 8
T_CORE = TOKENS // N_CORES  # 4096
TB = 512  # tokens per block
F32 = mybir.dt.float32
BF16 = mybir.dt.bfloat16


def build_program(t_core=T_CORE, hidden=HIDDEN, tb=TB, reps=1):
    """Build the per-core Bass program (SPMD: identical on all cores).
    reps>1 wraps the whole pipeline in a hardware For_i loop that recomputes
    the same output; used only for slope-based device timing."""
    nt = tb // 128          # token tiles per block
    nblk = t_core // tb     # blocks
    kc = hidden // 128      # contraction chunks
    nb = hidden // 512      # output column blocks
    assert tb % 128 == 0 and t_core % tb == 0 and hidden % 512 == 0

    nc = bacc.Bacc("TRN2", target_bir_lowering=False, debug=False)

    x_d = nc.dram_tensor("x", [t_core, hidden], F32, kind="ExternalInput").ap()
    w_d = [
        nc.dram_tensor(f"W{i}", [hidden, hidden], BF16, kind="ExternalInput").ap()
        for i in range(3)
    ]
    g_d = [
        nc.dram_tensor(f"g{i}", [hidden], F32, kind="ExternalInput").ap()
        for i in range(4)
    ]
    out_d = nc.dram_tensor("out", [t_core, hidden], F32, kind="ExternalOutput").ap()

    add = mybir.AluOpType.add
    relu = mybir.ActivationFunctionType.Relu
    sqrt = mybir.ActivationFunctionType.Sqrt
    square = mybir.ActivationFunctionType.Square

    with tile.TileContext(nc) as tc:
        with (
            tc.tile_pool(name="const", bufs=1) as const_pool,
            tc.tile_pool(name="resid", bufs=2) as resid_pool,
            tc.tile_pool(name="yhat", bufs=6) as yhat_pool,
            tc.tile_pool(name="yT", bufs=2) as yt_pool,
            tc.tile_pool(name="w", bufs=3) as w_pool,
            tc.tile_pool(name="small", bufs=12) as small_pool,
            tc.tile_pool(name="psum", bufs=8, space="PSUM") as psum_pool,
        ):
            eps_t = const_pool.tile([128, 1], F32)
            nc.vector.memset(eps_t, EPS)

            def bcast(ap):
                return bass.AP(
                    tensor=ap.tensor, offset=ap.offset, ap=[[0, 128]] + list(ap.ap)
                )

            g3t = const_pool.tile([128, hidden], F32, tag="g3")
            nc.gpsimd.dma_start(out=g3t, in_=bcast(g_d[3]))
            # bf16 broadcast copies of g0..g2 for block 0's unfolded path.
            gb = []
            for i in range(3):
                gt = const_pool.tile([128, hidden], BF16, tag=f"g{i}", name=f"gt{i}")
                nc.gpsimd.dma_start(out=gt, in_=bcast(g_d[i]))
                gb.append(gt)

            w_re = [w.rearrange("(kc p) n -> p kc n", p=128) for w in w_d]

            # Per-block pipeline state: (resid, ssp) keyed by block.
            state = {}

            def produce0(blk):
                """x load + relu (in place) + whole-row sum of squares."""
                resid = resid_pool.tile(
                    [128, nt, hidden], F32, tag="resid", name=f"resid{blk}"
                )
                ssp = [
                    small_pool.tile([128, nb], F32, tag=f"ssp{m}",
                                    name=f"ssp_b{blk}_{m}")
                    for m in range(nt)
                ]
                t0 = yt_pool.tile(
                    [128, nt * kc, 128], BF16, tag="yT", name=f"yt{blk}_0"
                )
                state[blk] = (resid, ssp)
                rss = []
                for m in range(nt):
                    nc.sync.dma_start(
                        out=resid[:, m, :],
                        in_=x_d[blk * tb + m * 128 : blk * tb + (m + 1) * 128, :],
                    )
                    nc.scalar.activation(
                        out=resid[:, m, :], in_=resid[:, m, :], func=relu
                    )
                    # Sum of squares per 512-column chunk (scratch into a
                    # short-lived SBUF tile).
                    scr = yhat_pool.tile(
                        [128, hidden], BF16, tag="yh", name=f"sqscr{blk}_{m}"
                    )
                    for n in range(nb):
                        nc.scalar.activation(
                            out=scr[:, n * 512 : (n + 1) * 512],
                            in_=resid[:, m, n * 512 : (n + 1) * 512],
                            func=square,
                            accum_out=ssp[m][:, n : n + 1],
                        )
                    # Emit this tile's stage-0 chain right away.
                    rss.append(chain_m(blk, 0, m, t0))
                return t0, rss

            def chain_m(blk, s, m, yt):
                """Emit token-tile m's boundary chain for stage s: rs from the
                accumulated squares, bf16 cast + gain, xbar transpose."""
                resid, ssp = state[blk]
                ss = small_pool.tile([128, 1], F32, tag="ss", name=f"ss{blk}_{s}_{m}")
                rs = small_pool.tile([128, 1], F32, tag="rs", name=f"rs{blk}_{s}_{m}")
                nc.vector.tensor_reduce(
                    ss, ssp[m], axis=mybir.AxisListType.X, op=add
                )
                nc.scalar.activation(
                    out=rs, in_=ss, func=sqrt, bias=eps_t[:, :], scale=1.0 / hidden
                )
                nc.vector.reciprocal(rs, rs)
                yh = yhat_pool.tile(
                    [128, hidden], BF16, tag="yh", name=f"yh{blk}_{s}_{m}"
                )
                nc.vector.tensor_scalar_mul(yh, resid[:, m, :], rs)
                nc.vector.tensor_mul(yh, yh, gb[s])
                nc.scalar.dma_start_transpose(
                    yt[:, m * kc : (m + 1) * kc, :], yh
                )
                return rs

            def mm_phase(blk, s, yt, rss, next_boundary):  # noqa: ARG001
                """resid += y_s @ W_s with incremental squares; if
                next_boundary is True (same-block stage s+1), emit each
                tile's next-stage chain at its final column block."""
                resid, _ = state[blk]
                wsrc = w_re[s]
                nssp = [
                    small_pool.tile([128, nb], F32, tag=f"ssp{m}",
                                    name=f"ssp_b{blk}_s{s}_{m}")
                    for m in range(nt)
                ]
                nyt = None
                nrss = []
                if next_boundary:
                    nyt = yt_pool.tile(
                        [128, nt * kc, 128], BF16, tag="yT", name=f"yt{blk}_{s + 1}"
                    )
                for n in range(nb):
                    wt = w_pool.tile(
                        [128, kc, 512], BF16, tag="w", name=f"w{blk}_{s}_{n}"
                    )
                    nc.sync.dma_start(
                        out=wt, in_=wsrc[:, :, n * 512 : (n + 1) * 512]
                    )
                    for m in range(nt):
                        ps = psum_pool.tile(
                            [128, 512], F32, tag="ps", name=f"ps{blk}_{s}_{n}_{m}"
                        )
                        for k in range(kc):
                            nc.tensor.matmul(
                                ps,
                                yt[:, m * kc + k, :],
                                wt[:, k, :],
                                start=(k == 0),
                                stop=(k == kc - 1),
                            )
                        rslice = resid[:, m, n * 512 : (n + 1) * 512]
                        nc.vector.tensor_add(rslice, rslice, ps)
                        # This slice of resid is now final for stage s+1:
                        # accumulate its squares (scratch into the now-dead
                        # psum tile).
                        nc.scalar.activation(
                            out=ps,
                            in_=rslice,
                            func=square,
                            accum_out=nssp[m][:, n : n + 1],
                        )
                        if n == nb - 1 and next_boundary:
                            state[blk] = (resid, nssp)
                            nrss.append(chain_m(blk, s + 1, m, nyt))
                state[blk] = (resid, nssp)
                return nyt, nrss

            def stage3_output(blk):
                """out = (resid * rs) * g3 in fp32, then store."""
                resid, ssp = state[blk]
                for m in range(nt):
                    ss = small_pool.tile([128, 1], F32, tag="ss", name=f"ss3_{blk}_{m}")
                    rs = small_pool.tile([128, 1], F32, tag="rs", name=f"rs3_{blk}_{m}")
                    nc.vector.tensor_reduce(
                        ss, ssp[m], axis=mybir.AxisListType.X, op=add
                    )
                    nc.scalar.activation(
                        out=rs, in_=ss, func=sqrt, bias=eps_t[:, :], scale=1.0 / hidden
                    )
                    nc.vector.reciprocal(rs, rs)
                    nc.vector.tensor_scalar_mul(resid[:, m, :], resid[:, m, :], rs)
                    nc.vector.tensor_mul(resid[:, m, :], resid[:, m, :], g3t)
                    nc.sync.dma_start(
                        out=out_d[blk * tb + m * 128 : blk * tb + (m + 1) * 128, :],
                        in_=resid[:, m, :],
                    )

            # ---- main pipeline ----
            def pipeline():
                t0, r0 = produce0(0)
                pending_out = None
                for blk in range(nblk):
                    t1, r1 = mm_phase(blk, 0, t0, r0, next_boundary=True)
                    if pending_out is not None:
                        stage3_output(pending_out)
                    t2, r2 = mm_phase(blk, 1, t1, r1, next_boundary=True)
                    if blk + 1 < nblk:
                        t0, r0 = produce0(blk + 1)
                    mm_phase(blk, 2, t2, r2, next_boundary=False)
                    pending_out = blk
                stage3_output(pending_out)

            if reps == 1:
                pipeline()
            else:
                with tc.For_i(0, reps, 1):
                    pipeline()

    nc.compile()
    return nc


_CACHE = {}


def _get_program(key=(T_CORE, HIDDEN, TB)):  # noqa: B008
    if key not in _CACHE:
        _CACHE[key] = build_program(*key)
    return _CACHE[key]


def run(inputs, trace=False):
    """Run on 8 NeuronCores. Returns (out, BassKernelResults)."""
    x = np.ascontiguousarray(np.asarray(inputs["x"], dtype=np.float32))
    ws = [
        np.ascontiguousarray(
            np.asarray(inputs[f"W{i}"], dtype=np.float32).astype(ml_dtypes.bfloat16)
        )
        for i in range(3)
    ]
    gs = [
        np.ascontiguousarray(np.asarray(inputs[f"g{i}"], dtype=np.float32))
        for i in range(4)
    ]

    nc = _get_program()
    in_maps = []
    for c in range(N_CORES):
        im = {"x": x[c * T_CORE : (c + 1) * T_CORE]}
        for i in range(3):
            im[f"W{i}"] = ws[i]
        for i in range(4):
            im[f"g{i}"] = gs[i]
        in_maps.append(im)

    res = run_bass_kernel_spmd(nc, in_maps, list(range(N_CORES)), trace=trace)
    out = np.concatenate([res.results[c]["out"] for c in range(N_CORES)], axis=0)
    return out, res


def kernel(**inputs) -> np.ndarray:
    out, _ = run(inputs, trace=False)
    return out
